# revision 33
# baseline (speedup 1.0000x reference)
"""GAT layer (4 heads, N=4096, E=131072) as a Trainium2 Bass/Tile SPMD kernel.

Row-partitioned (per sharding hint): core d owns destination rows
[d*512, (d+1)*512). Host preprocessing is index-only: dedup edges, bucket
by core / 128-row tile / 64-row half, pad each half to whole 128-slot
chunks, and emit the fp8 0/1 selection matrices M_T (edge->row) and
M_re (row->edge) plus wrapped gather-index tables.

Device phase A (replicated): h = nf16 @ W.T on PE (bf16); s-path from
the same bf16 nf via A2 split into bf16 hi/lo. h columns are stored
w-major (host-permuted W) so the per-edge v*g multiply on DVE hits the
2x packed mode. Fused he rows to DRAM are 256B each:
[h 120ch bf16 | h 8ch round-nearest int8 (W pre-scaled by 16, decoded
in the final reciprocal) | s_dst 4x fp16].

Phase B: 17 dma_gather calls of 1024 idxs (HW SWDGE carveout cap;
~853ns/call of Pool-engine descriptor generation is the stream floor;
256B rows keep the transfers fully hidden under it); s_src per edge via
PE matmuls against fp8 M_re (all lhsT/rhs at base partition 0 - the PE
corrupts in-flight matmuls when the stationary operand's base partition
alternates on real HW); v = exp(leaky_relu(s_src+s_dst)) in an fp16
chain, written straight into u's denominator columns; u = g*v with the
head dim innermost; aggregation + row sums via per-chunk fp8xbf16
matmuls into 64-row PSUM windows; normalize on ACT via per-partition
reciprocal scale, un-permuting columns.

M-matrix / index-table loads ride the otherwise-idle Pool DMA queue so
nf and he traffic own both HWDGE queues.

Known limit: a destination row with zero in-edges would produce NaN
(reference gives the uniform mean); impossible here (min degree 14).
"""

import numpy as np
import ml_dtypes

import concourse.bass as bass
import concourse.bacc as bacc
import concourse.mybir as mybir
import concourse.tile as tile
from concourse import bass_utils

F32 = mybir.dt.float32
BF16 = mybir.dt.bfloat16
FP16 = mybir.dt.float16
I8 = mybir.dt.int8
I16 = mybir.dt.int16
MAGIC = 12582912.0  # 1.5*2^23: f32 add forces round-to-nearest integer

N = 4096
CIN = 128
H = 4
CH = 32
FEAT = H * CH  # 128
NCORES = 8
RPC = N // NCORES  # 512 rows per core
RT = RPC // 128    # 4 row-tiles per core
NB = N // 128      # 32 node blocks
ALPHA = 0.2
EROW = 128         # he row: 128 bf16 slots = 256B
NBF = 120          # bf16 h channels per row (w-major positions 0:120)
PROWS = 64         # packed row window (PE base partition must be 0/32/64)
CALL_CHUNKS = 8    # 1024 idxs per dma_gather call (HW SWDGE carveout cap)

_BUILD_CACHE: dict[tuple, tuple] = {}


def _call_splits(C: int) -> tuple:
    """Split C chunks into call ranges of at most CALL_CHUNKS chunks."""
    bounds = list(range(0, C, CALL_CHUNKS))
    bounds.append(C)
    return tuple(bounds)


def _build(T: int, rlo: tuple = (), has_bias: bool = True):
    """Per-core program; T = chunks (of 128 edge slots) per 128-row tile."""
    import os
    KDEBUG = os.environ.get("KDEBUG", "") == "1"
    assert T % 2 == 0
    C = RT * T          # chunks per core
    L = C * 128         # edge slots per core
    cb = _call_splits(C)
    ncall = len(cb) - 1
    if not rlo:
        rlo = (0,) * C
    assert len(rlo) == C

    nc = bacc.Bacc("TRN2", target_bir_lowering=False, debug=False,
                   enable_asserts=False, num_devices=NCORES)

    # ---- I/O ----
    nf16_in = nc.dram_tensor("nf16", [CIN, N], BF16, kind="ExternalInput").ap()
    W_in = nc.dram_tensor("w", [FEAT, CIN], F32, kind="ExternalInput").ap()
    Wt16_in = nc.dram_tensor("wt16", [CIN, FEAT], BF16, kind="ExternalInput").ap()
    brow16_in = nc.dram_tensor("brow16", [1, FEAT], BF16, kind="ExternalInput").ap()
    bcol_in = nc.dram_tensor("bcol", [FEAT, 1], F32, kind="ExternalInput").ap()
    acat_in = nc.dram_tensor("acat", [FEAT, 8], F32, kind="ExternalInput").ap()
    mt_in = nc.dram_tensor("mt", [128, C, PROWS], FP8, kind="ExternalInput").ap()
    mre_in = nc.dram_tensor("mre", [PROWS, C, 128], FP8, kind="ExternalInput").ap()
    shf_in = nc.dram_tensor("shf", [128, PROWS], BF16, kind="ExternalInput").ap()
    gidx_in = nc.dram_tensor("gidx", [128, L // 16], I16, kind="ExternalInput").ap()
    bsel_in = nc.dram_tensor("bsel", [128, RT, NB], F32, kind="ExternalInput").ap()
    out_d = nc.dram_tensor("out", [RPC, FEAT], F32, kind="ExternalOutput").ap()
    if KDEBUG:
        dbg_ssrc = nc.dram_tensor("dbg_ssrc", [128, C, 4], F32,
                                  kind="ExternalOutput").ap()
        dbg_logit = nc.dram_tensor("dbg_logit", [128, C, 4], F32,
                                   kind="ExternalOutput").ap()
        dbg_sdst = nc.dram_tensor("dbg_sdst", [128, C, 4], F32,
                                  kind="ExternalOutput").ap()

    with tile.TileContext(nc) as tc:
        with (
            tc.tile_pool(name="const", bufs=1) as cp,
            tc.tile_pool(name="dram", bufs=1, space="DRAM") as dp,
            tc.tile_pool(name="work", bufs=2) as wp,
        ):
            # ---- tiny constants first (cheap, unblock everything) ----
            gidx_sb = cp.tile([128, L // 16], I16)
            nc.gpsimd.dma_start(out=gidx_sb[:], in_=gidx_in)
            W_sb = cp.tile([FEAT, CIN], F32)
            nc.scalar.dma_start(out=W_sb[:], in_=W_in)
            acat_sb = cp.tile([FEAT, 8], F32)
            nc.scalar.dma_start(out=acat_sb[:], in_=acat_in)
            bcol_sb = cp.tile([FEAT, 1], F32)
            nc.scalar.dma_start(out=bcol_sb[:], in_=bcol_in)
            Wt16_sb = cp.tile([CIN, FEAT], BF16)
            nc.sync.dma_start(out=Wt16_sb[:], in_=Wt16_in)
            brow16_sb = cp.tile([1, FEAT], BF16)
            nc.sync.dma_start(out=brow16_sb[:], in_=brow16_in)
            shf_sb = cp.tile([128, PROWS], BF16)
            nc.gpsimd.dma_start(out=shf_sb[:], in_=shf_in)
            bsel_sb = cp.tile([128, RT, NB], F32)
            nc.gpsimd.dma_start(out=bsel_sb[:], in_=bsel_in)
            ones_sb = cp.tile([1, FEAT], F32)
            nc.vector.memset(ones_sb[:], 1.0)
            ones16_sb = cp.tile([1, FEAT], BF16)
            nc.vector.memset(ones16_sb[:], 1.0)
            zero132_sb = cp.tile([128, FEAT + 4], F32)
            nc.vector.memset(zero132_sb[:], 0.0)
            ssk_sb = cp.tile([128, NB, 4], F32)   # s_src for all node blocks

            mre_sb = cp.tile([PROWS, C, 128], FP8)
            mt = cp.tile([128, C, PROWS], FP8)

            # ---- DRAM scratch ----
            he_dram = dp.tile([N, EROW], BF16)

            # ---- phase A (own pools; memory freed before phase B) ----
            with (
                tc.tile_pool(name="phA", bufs=1) as pa,
                tc.tile_pool(name="psA", bufs=4, space="PSUM") as psA,
                tc.tile_pool(name="psS", bufs=3, space="PSUM") as psS,
            ):
                nf16_sb = pa.tile([CIN, N], BF16, tag="nf16")
                # eighth-interleaved across both HWDGE queues so early
                # blocks unblock matmuls ASAP
                for q8 in range(8):
                    qs = slice(q8 * (N // 8), (q8 + 1) * (N // 8))
                    eng = nc.sync if q8 % 2 == 0 else nc.scalar
                    eng.dma_start(out=nf16_sb[:, qs], in_=nf16_in[:, qs])

                # A2 = W.T @ a_cat  (so s = nf @ A2 + b@a_cat)
                ps_a2 = psA.tile([FEAT, 8], F32, tag="psum_h")
                nc.tensor.matmul(ps_a2[:], lhsT=W_sb[:], rhs=acat_sb[:],
                                 start=True, stop=True)
                A2h_sb = cp.tile([CIN, 8], BF16)
                nc.vector.tensor_copy(out=A2h_sb[:], in_=ps_a2[:])
                A2l_sb = cp.tile([CIN, 8], BF16)
                nc.vector.tensor_tensor(out=A2l_sb[:], in0=ps_a2[:],
                                        in1=A2h_sb[:],
                                        op=mybir.AluOpType.subtract)
                ps_sb = psA.tile([1, 8], F32, tag="psum_h")
                nc.tensor.matmul(ps_sb[:], lhsT=bcol_sb[:], rhs=acat_sb[:],
                                 start=True, stop=True)
                sbias_sb = cp.tile([1, 8], BF16)
                nc.vector.tensor_copy(out=sbias_sb[:], in_=ps_sb[:])
                ones16b_sb = cp.tile([1, 8], BF16)
                nc.vector.memset(ones16b_sb[:], 1.0)

                he_big = pa.tile([128, NB, EROW], BF16, tag="hebig")
                for g in range(NB // 4):
                    ps_h = psA.tile([128, 4, FEAT], F32, tag="psum_h")
                    ps_s = psS.tile([128, 4, 8], F32, tag="psum_s")
                    for q in range(4):
                        nb = g * 4 + q
                        lhs16 = nf16_sb[:, nb * 128:(nb + 1) * 128]
                        nc.tensor.matmul(ps_h[:, q, :], lhsT=lhs16,
                                         rhs=Wt16_sb[:],
                                         start=True, stop=not has_bias)
                        if has_bias:
                            nc.tensor.matmul(ps_h[:, q, :],
                                             lhsT=ones16_sb[:],
                                             rhs=brow16_sb[:], start=False,
                                             stop=True)
                        # s = (hi+lo) @ (A2h+A2l) ~ hi@A2h + hi@A2l + lo@A2h
                        nc.tensor.matmul(ps_s[:, q, :], lhsT=lhs16,
                                         rhs=A2h_sb[:],
                                         start=True, stop=False)
                        nc.tensor.matmul(ps_s[:, q, :], lhsT=lhs16,
                                         rhs=A2l_sb[:],
                                         start=False, stop=not has_bias)
                        if has_bias:
                            nc.tensor.matmul(ps_s[:, q, :],
                                             lhsT=ones16_sb[:],
                                             rhs=sbias_sb[:], start=False,
                                             stop=True)
                    bs = slice(g * 4, (g + 1) * 4)
                    nc.scalar.copy(out=he_big[:, bs, 0:80],
                                   in_=ps_h[:, :, 0:80])
                    nc.vector.tensor_copy(out=he_big[:, bs, 80:NBF],
                                          in_=ps_h[:, :, 80:NBF])
                    # 8 overflow channels as round-nearest int8: Wt16 cols
                    # 120:128 are host-prescaled by 16, so a single
                    # (+MAGIC, -MAGIC) pass rounds to integer
                    he8 = he_big[:, bs, NBF:NBF + 4].bitcast(I8)
                    nc.vector.tensor_scalar(
                        out=he8, in0=ps_h[:, :, NBF:FEAT],
                        scalar1=MAGIC, scalar2=MAGIC,
                        op0=mybir.AluOpType.add, op1=mybir.AluOpType.subtract)
                    he16 = he_big[:, bs, NBF + 4:FEAT].bitcast(FP16)
                    nc.vector.tensor_copy(out=he16, in_=ps_s[:, :, 4:8])
                    nc.vector.tensor_copy(out=ssk_sb[:, bs, :],
                                          in_=ps_s[:, :, 0:4])
                    hed = he_dram[:].rearrange("(nb p) f -> p nb f", p=128)
                    eng = nc.sync if g % 2 == 0 else nc.scalar
                    with tc.high_priority():
                        eng.dma_start(out=hed[:, bs, :], in_=he_big[:, bs, :])

            # m-matrices on the Pool queue: it is idle before the gathers,
            # so these never displace nf/he traffic on the HWDGE queues
            for t4 in range(RT):
                ms = slice(t4 * T, (t4 + 1) * T)
                nc.gpsimd.dma_start(out=mre_sb[:, ms, :], in_=mre_in[:, ms, :])
            for m4 in range(4):
                ms = slice(m4 * (C // 4), (m4 + 1) * (C // 4))
                nc.gpsimd.dma_start(out=mt[:, ms, :], in_=mt_in[:, ms, :])

            # ---- phase B pools ----
            bp_cm = tc.tile_pool(name="big", bufs=1)
            bp = bp_cm.__enter__()
            psE_cm = tc.tile_pool(name="psE", bufs=1, space="PSUM")
            psE = psE_cm.__enter__()
            psO_cm = tc.tile_pool(name="psO", bufs=4, space="PSUM")
            psO = psO_cm.__enter__()

            # ---- gather he rows by dst, ~6 large calls ----
            gext = bp.tile([128, C, EROW], BF16, tag="gext")
            for j in range(ncall):
                k0, k1 = cb[j], cb[j + 1]
                nidx = (k1 - k0) * 128
                nc.gpsimd.dma_gather(
                    out_ap=gext[:, k0:k1, :],
                    in_ap=he_dram[:],
                    idxs_ap=gidx_sb[:, k0 * 8:k1 * 8],
                    num_idxs=nidx, num_idxs_reg=nidx, elem_size=EROW)

            # ---- s_src per edge: PE expansion via M_re ----
            ssrc = bp.tile([128, C, 4], FP16, tag="ssrc")
            for t in range(RT):
                # select this row-tile's s rows: sum_b bsel[t,b]*ssk[:,b,:]
                stile_t = wp.tile([128, 4, NB], F32, tag="stile_t", bufs=1)
                nc.vector.tensor_tensor(
                    out=stile_t[:],
                    in0=ssk_sb[:].rearrange("p b f -> p f b"),
                    in1=bsel_sb[:, t, :][:, None, :].to_broadcast(
                        [128, 4, NB]),
                    op=mybir.AluOpType.mult)
                srow = wp.tile([128, 4], F32, tag="srow", bufs=1)
                nc.vector.tensor_reduce(
                    out=srow[:, :, None], in_=stile_t[:],
                    op=mybir.AluOpType.add, axis=mybir.AxisListType.X)
                # bf16 hi/lo split so the matmul operands are all bf16
                srow_h = wp.tile([128, 4], BF16, tag="srow_h", bufs=1)
                nc.vector.tensor_copy(out=srow_h[:], in_=srow[:])
                srow_l = wp.tile([128, 4], BF16, tag="srow_l", bufs=1)
                nc.vector.tensor_tensor(out=srow_l[:], in0=srow[:],
                                        in1=srow_h[:],
                                        op=mybir.AluOpType.subtract)
                # rows 64:128 shifted to base partition 0 via PE
                ps_sh = psE.tile([PROWS, 8], F32, tag="psum_sh")
                nc.tensor.matmul(ps_sh[:, 0:4], lhsT=shf_sb[:],
                                 rhs=srow_h[:], start=True, stop=True)
                nc.tensor.matmul(ps_sh[:, 4:8], lhsT=shf_sb[:],
                                 rhs=srow_l[:], start=True, stop=True)
                shi_h = wp.tile([PROWS, 4], BF16, tag="shi_h", bufs=1)
                nc.vector.tensor_copy(out=shi_h[:], in_=ps_sh[:, 0:4])
                shi_l = wp.tile([PROWS, 4], BF16, tag="shi_l", bufs=1)
                nc.vector.tensor_copy(out=shi_l[:], in_=ps_sh[:, 4:8])
                for half in range((T + 63) // 64):
                    base = t * T + half * 64
                    nchunk = min(64, (t + 1) * T - base)
                    ps_x = psE.tile([128, 256], F32, tag="psum_e")
                    for cc in range(nchunk):
                        k = base + cc
                        if rlo[k] == 0:
                            r1, r2 = srow_h[0:PROWS, :], srow_l[0:PROWS, :]
                        else:
                            r1, r2 = shi_h[:], shi_l[:]
                        nc.tensor.matmul(ps_x[:, cc * 4:(cc + 1) * 4],
                                         lhsT=mre_sb[:, k, :], rhs=r1,
                                         start=True, stop=False)
                        nc.tensor.matmul(ps_x[:, cc * 4:(cc + 1) * 4],
                                         lhsT=mre_sb[:, k, :], rhs=r2,
                                         start=False, stop=True)
                    nc.scalar.copy(
                        out=ssrc[:, base:base + nchunk, :],
                        in_=ps_x[:, 0:4 * nchunk].rearrange(
                            "p (c f) -> p c f", f=4))

            if KDEBUG:
                nc.sync.dma_start(out=dbg_ssrc, in_=ssrc[:])

            # ---- per gather call: v chain + u = g*v ----
            u = bp.tile([128, C, FEAT + 4], BF16, tag="u")
            for j in range(ncall):
                k0, k1 = cb[j], cb[j + 1]
                cc = k1 - k0
                sdst16 = gext[:, k0:k1, NBF + 4:FEAT].bitcast(FP16)
                logit = wp.tile([128, cc, 4], FP16, tag="logit", bufs=3)
                nc.vector.tensor_tensor(out=logit[:], in0=ssrc[:, k0:k1, :],
                                        in1=sdst16,
                                        op=mybir.AluOpType.add)
                if KDEBUG:
                    sdf = wp.tile([128, cc, 4], F32, tag="sdf", bufs=2)
                    nc.vector.tensor_copy(out=sdf[:], in_=sdst16)
                    nc.sync.dma_start(out=dbg_sdst[:, k0:k1, :], in_=sdf[:])
                nc.vector.scalar_tensor_tensor(
                    out=logit[:], in0=logit[:], scalar=ALPHA, in1=logit[:],
                    op0=mybir.AluOpType.mult, op1=mybir.AluOpType.max)
                if KDEBUG:
                    nc.sync.dma_start(out=dbg_logit[:, k0:k1, :], in_=logit[:])
                # exp straight into u's denominator columns
                nc.scalar.activation(out=u[:, k0:k1, FEAT:FEAT + 4],
                                     in_=logit[:],
                                     func=mybir.ActivationFunctionType.Exp)
                # u = g*v, head dim innermost (2x DVE packed mode)
                nc.vector.tensor_tensor(
                    out=u[:, k0:k1, 0:NBF].rearrange(
                        "p c (w h) -> p c w h", h=H),
                    in0=gext[:, k0:k1, 0:NBF].rearrange(
                        "p c (w h) -> p c w h", h=H),
                    in1=u[:, k0:k1, FEAT:FEAT + 4][:, :, None, :].to_broadcast(
                        [128, cc, NBF // H, H]),
                    op=mybir.AluOpType.mult)
                g8 = gext[:, k0:k1, NBF:NBF + 4].bitcast(I8)
                nc.vector.tensor_tensor(
                    out=u[:, k0:k1, NBF:FEAT].rearrange(
                        "p c (w h) -> p c w h", h=H),
                    in0=g8.rearrange("p c (w h) -> p c w h", h=H),
                    in1=u[:, k0:k1, FEAT:FEAT + 4][:, :, None, :].to_broadcast(
                        [128, cc, 2, H]),
                    op=mybir.AluOpType.mult)

            # ---- aggregation + normalize per row-tile ----
            for t in range(RT):
                ps_o = psO.tile([128, FEAT + 4], F32, tag="psum_o")
                nc.scalar.copy(out=ps_o[:], in_=zero132_sb[:])
                for c in range(T):
                    k = t * T + c
                    rl = rlo[k]
                    nc.tensor.matmul(ps_o[rl:rl + PROWS, :],
                                     lhsT=mt[:, k, :], rhs=u[:, k, :],
                                     start=False, stop=(c == T - 1),
                                     skip_group_check=True)
                den_sb = wp.tile([128, 4], F32, tag="den")
                with tc.high_priority():
                    nc.vector.tensor_copy(out=den_sb[:],
                                          in_=ps_o[:, FEAT:FEAT + 4])
                rec_sb = wp.tile([128, 8], F32, tag="rec")
                nc.vector.reciprocal(out=rec_sb[:, 0:4], in_=den_sb[:])
                # int8-decode factor folded into the w30/w31 reciprocal
                nc.vector.tensor_scalar(
                    out=rec_sb[:, 4:8], in0=rec_sb[:, 0:4],
                    scalar1=2.0 ** -4, scalar2=None,
                    op0=mybir.AluOpType.mult)
                o_sb = wp.tile([128, FEAT], F32, tag="osb")
                psv = ps_o[:, 0:FEAT].rearrange("p (w h) -> p w h", h=H)
                with tc.high_priority():
                    for hh in range(H):
                        nc.scalar.activation(
                            out=o_sb[:, hh * CH:hh * CH + 30],
                            in_=psv[:, 0:30, hh],
                            func=mybir.ActivationFunctionType.Copy,
                            scale=rec_sb[:, hh:hh + 1])
                        nc.scalar.activation(
                            out=o_sb[:, hh * CH + 30:(hh + 1) * CH],
                            in_=psv[:, 30:32, hh],
                            func=mybir.ActivationFunctionType.Copy,
                            scale=rec_sb[:, 4 + hh:5 + hh])
                eng = nc.sync if t % 2 == 0 else nc.scalar
                eng.dma_start(out=out_d[t * 128:(t + 1) * 128, :],
                              in_=o_sb[:])

            psO_cm.__exit__(None, None, None)
            psE_cm.__exit__(None, None, None)
            bp_cm.__exit__(None, None, None)

    nc.compile()
    return nc


def _get_build(T: int, rlo: tuple, has_bias: bool):
    key = (T, rlo, has_bias)
    if key not in _BUILD_CACHE:
        _BUILD_CACHE[key] = _build(T, rlo, has_bias)
    return _BUILD_CACHE[key]


def _wrap_gather_idx(idx: np.ndarray, L: int, cb: tuple) -> np.ndarray:
    """Pack index list into the [128, L/16] int16 layout dma_gather wants:
    per call j covering idxs [s, e), index i of that call at
    [i % 16, s//16 + i // 16], replicated across the 8 16-partition groups."""
    out = np.zeros((128, L // 16), np.int16)
    for j in range(len(cb) - 1):
        s, e = cb[j] * 128, cb[j + 1] * 128
        n = e - s
        blk = idx[s:e].astype(np.int16).reshape(n // 16, 16).T
        for c in range(8):
            out[16 * c:16 * (c + 1), s // 16:e // 16] = blk
    return out


# stored h position c' = w*4+h  <->  original channel h*32+w
_PERM = np.array([(c % H) * CH + c // H for c in range(FEAT)], np.int64)


def kernel(**inputs) -> np.ndarray:
    node_feats = np.asarray(inputs["node_feats"], dtype=np.float32)
    W = np.asarray(inputs["W"], dtype=np.float32)
    b = np.asarray(inputs["b"], dtype=np.float32)
    a = np.asarray(inputs["a"], dtype=np.float32)
    edge_index = np.asarray(inputs["edge_index"])

    src = edge_index[0].astype(np.int64)
    dst = edge_index[1].astype(np.int64)
    # dedup (matches dense .at[].set semantics; duplicate logits identical)
    keys = np.unique(src * N + dst)
    su = (keys // N).astype(np.int64)
    du = (keys % N).astype(np.int64)

    # sort edges by (owning 64-row half, dst)
    order = np.lexsort((du, su // 64))
    su = su[order]
    du = du[order]
    half_id = su // 64  # 64 halves
    hcounts = np.bincount(half_id, minlength=N // 64)
    hstarts = np.zeros(N // 64 + 1, np.int64)
    np.cumsum(hcounts, out=hstarts[1:])
    hchunks = -(-hcounts // 128)          # chunks per half

    # shared program: pad both halves to fixed chunk counts (max over cores)
    h0 = hchunks[0::2].reshape(NCORES, RT)
    h1 = hchunks[1::2].reshape(NCORES, RT)
    n0 = int(h0.max())
    n1 = int(h1.max())
    T = n0 + n1 + ((n0 + n1) % 2)
    C = RT * T
    L = C * 128
    rlo = np.zeros(C, np.int64)
    for t in range(RT):
        rlo[t * T:t * T + n0] = 0
        rlo[t * T + n0:t * T + n0 + n1] = 64
    rlo_t = tuple(int(x) for x in rlo)

    nc = _get_build(T, rlo_t, bool(np.any(b)))
    cb = _call_splits(C)

    # constant marshalling (index shuffles only, no FP math)
    a_cat = np.zeros((FEAT, 8), np.float32)
    for hh in range(H):
        a_cat[hh * CH:(hh + 1) * CH, hh] = a[hh, :CH]
        a_cat[hh * CH:(hh + 1) * CH, 4 + hh] = a[hh, CH:]
    nf_T = np.ascontiguousarray(node_feats.T)
    nf16 = nf_T.astype(ml_dtypes.bfloat16)
    W_perm = np.ascontiguousarray(W[_PERM, :])
    b_perm = b[_PERM]
    Wt16f = np.ascontiguousarray(W_perm.T).astype(np.float32)
    brow16f = b_perm.reshape(1, FEAT).astype(np.float32)
    Wt16f[:, NBF:] *= 16.0
    brow16f[:, NBF:] *= 16.0
    Wt16 = Wt16f.astype(ml_dtypes.bfloat16)
    brow16 = brow16f.astype(ml_dtypes.bfloat16)
    bcol = b.reshape(FEAT, 1)
    jj = np.arange(PROWS)
    shf = (np.arange(128)[:, None] == (jj[None, :] + 64)).astype(
        ml_dtypes.bfloat16)

    in_maps = []
    for d in range(NCORES):
        gidx = np.zeros(L, np.int64)
        srel = np.full((128, C), -999.0, np.float32)   # shifted by rlo
        bsel = np.zeros((128, RT, NB), np.float32)
        for t in range(RT):
            gt = RT * d + t
            bsel[:, t, gt] = 1.0
            for hh, base_c, nch in ((0, 0, n0), (1, n0, n1)):
                hid = gt * 2 + hh
                lo, n_e = hstarts[hid], hcounts[hid]
                rel = np.full(nch * 128, -1.0, np.float32)
                rel[:n_e] = (su[lo:lo + n_e] - gt * 128).astype(np.float32)
                cs = t * T + base_c
                srel[:, cs:cs + nch] = rel.reshape(nch, 128).T - 64 * hh
                gi = np.zeros(nch * 128, np.int64)
                gi[:n_e] = du[lo:lo + n_e]
                gidx[cs * 128:(cs + nch) * 128] = gi
        mt = (srel[:, :, None] == jj[None, None, :]).astype(
            ml_dtypes.float8_e4m3fn)
        mre = np.ascontiguousarray(mt.transpose(2, 1, 0))  # [64, C, 128]
        in_maps.append({
            "nf16": nf16, "w": W, "wt16": Wt16,
            "brow16": brow16, "bcol": bcol,
            "acat": a_cat, "mre": mre,
            "mt": np.ascontiguousarray(mt),
            "gidx": _wrap_gather_idx(gidx, L, cb), "bsel": bsel, "shf": shf,
        })

    res = bass_utils.run_bass_kernel_spmd(nc, in_maps,
                                          core_ids=list(range(NCORES)))
    out = np.concatenate([res.results[d]["out"] for d in range(NCORES)],
                         axis=0)
    return np.ascontiguousarray(out.astype(np.float32))
